# revision 55
# baseline (speedup 1.0000x reference)
"""Causal single-head attention (B=8, T=2048, C=1024, H=64) on 8 trn2 NeuronCores.

Strategy (data-parallel over batch, one batch element per core):
  host: feed xT = x[b].T in fp16 (C on partitions = contraction dim; halves
        the dominant DMA), wqk = [Wq | Wk] fused projection weight, wv = Wv
        (duplicated to [Wv | Wv] on-chip).
  core, per q-block of 512 tokens:
    proj: per C-chunk, a fused [Wq|Wk] matmul (psum rows 0:64 = qT block,
          rows 64:128 = kT) interleaved with a [Wv|Wv] M=128 matmul whose
          output lands row-duplicated, so the v transposes below can
          alternate PE row halves and co-issue in pairs.
    vT -> v: PE transposes (row-half alternating) + a ones column kept in
          v_s via a one-time memset (softmax denominator accumulates through
          the PV matmul's 65th output row).
    QK:   scores^T[s,q] per 128-wide s-chunk; chunk parity alternates PE
          row-halves so pairs co-issue.
    exp:  one ACT op per chunk-pair reads two adjacent psum banks [128,1024]
          -> pT (fp16), 1/sqrt(C) applied via ACT's free affine pre-scale.
          No max subtraction (scores/sqrt(C) are O(1): exp is overflow-safe).
    mask: triangular 128x128 multiply on the diagonal square (DVE); columns
          left of the causal frontier are never read by PV.
    PV:   out_aug^T[65, q] += v_aug-stationary @ pT-moving (causal widths).
          Final block drains psO stripe-by-stripe into the out DMA.
  host: out = (out_aug[:64] / out_aug[64]).T, stack cores.

fp16 on the PE (full rate, FWL weight loads, half DMA); fp32 PSUM accum.
Feature flags (KFLAGS env) gate individual optimizations; defaults are the
measured-best set.  Bisection notes: flags a,b,c,e,h,i,j,k,l,n,o,q,s,t all
regressed on hardware (the Tile scheduler punishes added cross-engine
coupling and non-uniform matmul shapes; notably s proved the QK row-half
alternation is worth ~8us — same-row-group QK weight loads do NOT pipeline);
d,f,m are the keepers:
  d: psB M=128 + paired transposes   f: ones-column memset
  m: less head DMA (Wv once + tri trimmed)
HW exec ~58.8-62.5us, mean ~60.4 (baseline 62.6-63.1); err ~4.7e-4.
"""

import os
import numpy as np

import concourse.bass as bass
import concourse.mybir as mybir
import concourse.tile as tile
from concourse import bacc
from concourse.bass_utils import run_bass_kernel_spmd

B, T, C, H = 8, 2048, 1024, 64
TB = 512                 # q-block width
NBLK = T // TB           # 4 q-blocks
NC = C // 128            # 8 contraction chunks
NSC = T // 128           # 16 s-chunks
HA = H + 1               # v augmented with ones column
F32 = mybir.dt.float32
F16 = mybir.dt.float16

# --- feature flags (bisection; default = measured-best set) ---
FLAGS = os.environ.get("KFLAGS", "dfm")
F_ENG = "a" in FLAGS     # out-copy on DVE (regression; keep off)
F_QKC = "b" in FLAGS     # causal-narrow QK matmuls (regression; keep off)
F_BIGX = "c" in FLAGS    # blocks 1-3 x as one 1MiB dma_start (regression)
F_PSB = "d" in FLAGS     # psB M=128 ([Wv|Wv]) + row-alternating transposes
F_MASK = "e" in FLAGS    # causal mask via PE matmul (regression; keep off)
F_ONES = "f" in FLAGS    # v_s ones column pre-memset (drops 16 tiny copies)
F_QUAD = "g" in FLAGS    # one exp ACT per 4 s-chunks (manual psQK halves)
F_XSPL = "h" in FLAGS    # x chunks of blocks 1,3 issued on the scalar ring
F_VTIL = "i" in FLAGS    # transposes for chunks 2,3 emitted after group 0 so
                         # the first QK pair isn't stuck behind ps_vt slots
F_QKSP = "j" in FLAGS    # qk dup copies split across DVE+ACT on early blocks
F_X2 = "k" in FLAGS      # x delivered as 2-chunk (256KB) dma_starts (blocks 1-3)
F_DUP2 = "l" in FLAGS    # q/k dup halves via cheap fp16 SBUF->SBUF copies
F_WDIET = "m" in FLAGS   # Wv sent once (dup on-chip) + tri trimmed: less head DMA
F_OSPL = "n" in FLAGS    # final-block stripe out-DMAs alternate sync/gpsimd
F_ACTN = "o" in FLAGS    # exp ACT skips fully-dead leading cols on diag pairs
F_XRNG = "q" in FLAGS    # block-0 x chunks split across sync+scalar rings
F_QK1 = "s" in FLAGS     # QK on row-half h0 only: no q/k dup copies (2 not 4
                         # per block); forgoes QK co-issue (~23ns/matmul)
F_VACT = "t" in FLAGS    # block-0 vT2/v_s copies on the (idle-until-24us) ACT
F_VTSL = ("v" in FLAGS) or ("w" in FLAGS)
                         # v: psA bufs=1 + full-bank dedicated ps_vt slot
                         # w: psA bufs=2 + sub-bank ps_vt slot (if packing OK)

_compiled = {}


def build_nc():
    nc = bacc.Bacc("TRN2", target_bir_lowering=False, debug=False, num_devices=8)

    WVW = 64 if F_WDIET else 128
    TRIW = 129 if (F_WDIET and not F_MASK) else 385
    xT_d = nc.dram_tensor("xT", [C, T], F16, kind="ExternalInput").ap()
    wqk_d = nc.dram_tensor("wqk", [C, 128], F16, kind="ExternalInput").ap()
    wv_d = nc.dram_tensor("wv", [C, WVW], F16, kind="ExternalInput").ap()
    # col 0:128 = upper-tri keep-mask (1 incl. diagonal); col 128 = ones;
    # col 129:257 = strict-upper 0/1 (mask L); col 257:385 = -30000 * I
    tri_d = nc.dram_tensor("tri", [128, TRIW], F16, kind="ExternalInput").ap()
    outT_d = nc.dram_tensor("outT", [HA, T], F32, kind="ExternalOutput").ap()

    xT_r = xT_d.rearrange("(co ci) t -> ci co t", ci=128)
    wqk_r = wqk_d.rearrange("(co ci) m -> ci co m", ci=128)
    wv_r = wv_d.rearrange("(co ci) m -> ci co m", ci=128)

    with tile.TileContext(nc) as tc:
        with (
            tc.tile_pool(name="const", bufs=1) as cpool,
            tc.tile_pool(name="persist", bufs=1) as ppool,
            tc.tile_pool(name="xin", bufs=8 if F_BIGX else 32) as xpool,
            tc.tile_pool(name="xblk", bufs=3) as xbpool,
            tc.tile_pool(name="ptile", bufs=6 if "x" in FLAGS else 4) as pt_pool,
            tc.tile_pool(name="vtmp", bufs=3 if "x" in FLAGS else 2) as vtmp_pool,
            tc.tile_pool(name="outsb", bufs=3 if "x" in FLAGS else 2) as out_pool,
            tc.tile_pool(
                name="psA", bufs=1 if FLAGS.count("v") else 2, space="PSUM"
            ) as psA_pool,
            tc.tile_pool(name="psB", bufs=1, space="PSUM") as psB_pool,
            tc.tile_pool(name="psQK", bufs=2, space="PSUM") as psQK_pool,
            tc.tile_pool(name="psO", bufs=1, space="PSUM") as psO_pool,
        ):
            # constants / weights on the scalar HWDGE ring so they don't queue
            # behind the x stream on the sync ring
            wqk_s = cpool.tile([128, NC, 128], F16)
            wv_s = cpool.tile([128, NC, 128], F16)
            tri_full = cpool.tile([128, TRIW], F16)
            tri_s = tri_full[:, 0:128]
            ones_s = tri_full[:, 128:129]
            if F_MASK:
                maskL = tri_full[:, 129:257]
                maskU = tri_full[:, 257:385]
            ident = cpool.tile([128, 64], F16)
            # chunk 0 first so the very first matmul's weights land ASAP
            nc.scalar.dma_start(wqk_s[:, 0:1, :], wqk_r[:, 0:1, :])
            nc.scalar.dma_start(wqk_s[:, 1:NC, :], wqk_r[:, 1:NC, :])
            if F_WDIET:
                # Wv arrives once; the [Wv|Wv] dup is a cheap on-chip copy
                nc.scalar.dma_start(wv_s[:, :, 0:64], wv_r[:])
                if F_PSB:
                    nc.vector.tensor_copy(wv_s[:, :, 64:128], wv_s[:, :, 0:64])
            else:
                nc.scalar.dma_start(wv_s[:], wv_r[:])
            nc.scalar.dma_start(tri_full[:], tri_d[:])

            # PE warm-up: dummy self-contained matmuls with no DMA deps keep
            # the PE busy through the initial DMA wait so the HAM clock gate
            # reaches K=8/8 before real work arrives.
            warm_w = cpool.tile([128, 128], F32)
            warm_x = cpool.tile([128, 512], F32)
            nc.vector.memset(warm_w[:], 0.0)
            nc.vector.memset(warm_x[:], 0.0)

            def warm_mm(n):
                ps_warm = psQK_pool.tile([128, 512], F32, tag="psQK")
                nc.tensor.matmul(ps_warm[:, 0:n], warm_w[:], warm_x[:, 0:n],
                                 start=True, stop=True)

            # bridge the initial DMA wait (fp32 = 2 passes, long dense ops)
            for w in range(2):
                warm_mm(512)
            for h2 in range(2):
                sl = ident[h2 * 64 : (h2 + 1) * 64, :]
                nc.gpsimd.memset(sl, 0.0)
                nc.gpsimd.affine_select(
                    out=sl,
                    in_=sl,
                    compare_op=mybir.AluOpType.not_equal,
                    fill=1.0,
                    base=0,
                    pattern=[[-1, 64]],
                    channel_multiplier=1,
                )

            # row-duplicated q/k (rows 0:64 == rows 64:128) for row-half
            # alternating QK stationaries/movings
            qT2_s = ppool.tile([128, T], F16)
            kT2_s = ppool.tile([128, T], F16)
            v_s = ppool.tile([128, NSC * HA], F16)
            if F_ONES:
                # ones columns (col 64 of each 65-block) pre-set once
                nc.vector.memset(v_s[:], 1.0)

            for i in range(NBLK):
                q0 = i * TB
                # ---- x DMA for this block ----
                if F_X2 and i > 0:
                    # 2-chunk tiles: ~halves the per-byte dma_start issue cost
                    # (issue time is near-constant per op) at 256KB sem grain
                    x_c = []
                    for c2 in range(NC // 2):
                        xp = xpool.tile([128, 2, TB], F16, tag="x2")
                        nc.sync.dma_start(
                            xp[:], xT_r[:, 2 * c2 : 2 * c2 + 2, q0 : q0 + TB]
                        )
                        x_c.extend([xp[:, 0, :], xp[:, 1, :]])
                elif not F_BIGX or i == 0:
                    x_c = []
                    xeng = nc.scalar if (F_XSPL and i % 2 == 1) else nc.sync
                    for c in range(NC):
                        xc = xpool.tile([128, TB], F16)
                        if i == 0 and c == 0:
                            # split so the first 128 cols land (and the first
                            # matmul can start) as early as possible
                            nc.sync.dma_start(xc[:, 0:128], xT_r[:, c, q0 : q0 + 128])
                            nc.sync.dma_start(
                                xc[:, 128:TB], xT_r[:, c, q0 + 128 : q0 + TB]
                            )
                        else:
                            if F_XRNG and i >= 1 and c % 2 == 1:
                                # odd chunks ride the scalar ring: the two
                                # HWDGE rings' bandwidths aggregate, so x0
                                # (which gates everything) lands sooner
                                nc.scalar.dma_start(xc[:], xT_r[:, c, q0 : q0 + TB])
                            else:
                                xeng.dma_start(xc[:], xT_r[:, c, q0 : q0 + TB])
                        x_c.append(xc)
                else:
                    xb = xbpool.tile([128, NC, TB], F16)
                    eng = nc.scalar if i == 1 else nc.sync
                    eng.dma_start(xb[:], xT_r[:, :, q0 : q0 + TB])
                    x_c = [xb[:, c, :] for c in range(NC)]

                # ---- passA+passB interleaved per C-chunk ----
                psA = psA_pool.tile([128, TB], F32)
                psB = psB_pool.tile([128 if F_PSB else 64, TB], F32)
                for c in range(NC):
                    if i == 0 and c == 0:
                        nc.tensor.matmul(
                            psA[:, 0:128], wqk_s[:, 0, :], x_c[0][:, 0:128],
                            start=True, stop=False,
                        )
                        nc.tensor.matmul(
                            psA[:, 128:TB], wqk_s[:, 0, :], x_c[0][:, 128:TB],
                            start=False, stop=False,
                        )
                    else:
                        nc.tensor.matmul(
                            psA[:], wqk_s[:, c, :], x_c[c][:],
                            start=(c == 0 and i != 0), stop=(c == NC - 1),
                        )
                    nc.tensor.matmul(
                        psB[:], wv_s[:, c, :] if F_PSB else wv_s[:, c, 0:64],
                        x_c[c][:],
                        start=(c == 0), stop=(c == NC - 1),
                    )
                if F_QKSP and i <= 1:
                    # ACT is idle until the first exp (~24us): let it do half
                    # the dup copies so all four finish ~1.4us sooner
                    nc.vector.tensor_copy(qT2_s[0:64, q0 : q0 + TB], psA[0:64, :])
                    nc.scalar.copy(kT2_s[0:64, q0 : q0 + TB], psA[64:128, :])
                    nc.scalar.copy(qT2_s[64:128, q0 : q0 + TB], psA[0:64, :])
                    nc.vector.tensor_copy(kT2_s[64:128, q0 : q0 + TB], psA[64:128, :])
                elif F_QK1:
                    # all QK matmuls live on row-half h0: exactly two copies
                    nc.vector.tensor_copy(qT2_s[0:64, q0 : q0 + TB], psA[0:64, :])
                    nc.vector.tensor_copy(kT2_s[0:64, q0 : q0 + TB], psA[64:128, :])
                elif F_DUP2:
                    # PSUM read once per tensor; the dup half is a cheap fp16
                    # SBUF->SBUF copy (2x/4x DVE mode) and psA frees sooner
                    nc.vector.tensor_copy(qT2_s[0:64, q0 : q0 + TB], psA[0:64, :])
                    nc.vector.tensor_copy(kT2_s[64:128, q0 : q0 + TB], psA[64:128, :])
                    nc.vector.tensor_copy(
                        qT2_s[64:128, q0 : q0 + TB], qT2_s[0:64, q0 : q0 + TB]
                    )
                    nc.vector.tensor_copy(
                        kT2_s[0:64, q0 : q0 + TB], kT2_s[64:128, q0 : q0 + TB]
                    )
                else:
                    for h2 in range(2):
                        r = slice(h2 * 64, h2 * 64 + 64)
                        nc.vector.tensor_copy(qT2_s[r, q0 : q0 + TB], psA[0:64, :])
                        nc.vector.tensor_copy(kT2_s[r, q0 : q0 + TB], psA[64:128, :])

                vT2 = vtmp_pool.tile([128 if F_PSB else 64, TB], F16)
                vceng = nc.scalar if (F_VACT and i == 0) else nc.vector
                if vceng is nc.scalar:
                    vceng.copy(vT2[:], psB[:])
                else:
                    vceng.tensor_copy(vT2[:], psB[:])

                def emit_vt(j4):
                    sj = (TB // 128) * i + j4
                    if F_PSB:
                        r = slice((j4 % 2) * 64, (j4 % 2) * 64 + 64)
                    else:
                        r = slice(0, 64)
                    if "v" in FLAGS:
                        ps_vt = psQK_pool.tile(
                            [128, 64], F16, tag="vt", bufs=1,
                            padded_shape=[128, 1024],  # full-bank aligned
                        )
                    elif "w" in FLAGS:
                        ps_vt = psQK_pool.tile([128, 64], F16, tag="vt", bufs=1)
                    else:
                        ps_vt = psQK_pool.tile([128, 64], F16, tag="psQK")
                    nc.tensor.transpose(
                        ps_vt[:],
                        vT2[r, j4 * 128 : (j4 + 1) * 128],
                        ident[r, :],
                    )
                    if vceng is nc.scalar:
                        vceng.copy(v_s[:, sj * HA : sj * HA + H], ps_vt[:])
                    else:
                        vceng.tensor_copy(v_s[:, sj * HA : sj * HA + H], ps_vt[:])
                    if not F_ONES:
                        nc.vector.tensor_copy(
                            v_s[:, sj * HA + H : sj * HA + HA], ones_s[:]
                        )

                for j4 in range(2 if F_VTIL else TB // 128):
                    emit_vt(j4)

                # ---- attention for this q-block ----
                nsc_i = (TB // 128) * (i + 1)  # s-chunks 0..nsc_i-1 (causal)
                psO = psO_pool.tile([HA, TB], F32)
                if i == NBLK - 1:
                    out_sb_last = out_pool.tile([HA, TB], F32)
                for g in range(nsc_i // 2):  # pairs of s-chunks share one exp
                    js = [2 * g, 2 * g + 1]
                    ds = [j * 128 - q0 for j in js]
                    los = [max(dd, 0) for dd in ds]
                    psQK = psQK_pool.tile([128, 1024], F32, tag="psQK")
                    for h2 in range(2):
                        j, d, lo = js[h2], ds[h2], los[h2]
                        if F_QK1:
                            r = slice(0, 64)  # single row half, no dup needed
                        else:
                            r = slice(h2 * 64, h2 * 64 + 64)  # alternate halves
                        qklo = lo if F_QKC else 0
                        nc.tensor.matmul(
                            psQK[:, h2 * 512 + qklo : (h2 + 1) * 512],
                            kT2_s[r, j * 128 : (j + 1) * 128],
                            qT2_s[r, q0 + qklo : q0 + TB],
                            start=True, stop=not (F_MASK and d >= 0),
                        )
                        if F_MASK and d >= 0:
                            # diagonal: add -30000 onto causally-dead (q < s)
                            # scores; exp then lands exact zeros, no mask op
                            nc.tensor.matmul(
                                psQK[:, h2 * 512 + d : h2 * 512 + d + 128],
                                maskL[:], maskU[:],
                                start=False, stop=True,
                            )
                    pT = pt_pool.tile([128, 1024], F16)
                    # cols < los[0] of the first chunk are causally dead for
                    # the whole pair: skip them in the exp (PV never reads)
                    alo = los[0] if F_ACTN else 0
                    nc.scalar.activation(
                        pT[:, alo:1024], psQK[:, alo:1024],
                        mybir.ActivationFunctionType.Exp,
                        scale=float(1.0 / np.sqrt(C)),
                    )
                    if F_VTIL and g == 0:
                        # late chunks' transposes slot in behind the first QK
                        # pair (their v is only read by this block's last PVs)
                        emit_vt(2)
                        emit_vt(3)
                    for h2 in range(2):
                        j, d, lo = js[h2], ds[h2], los[h2]
                        pj = pT[:, h2 * 512 : (h2 + 1) * 512]
                        if not F_MASK and d >= 0:
                            nc.vector.tensor_mul(
                                pj[:, d : d + 128], pj[:, d : d + 128], tri_s[:]
                            )
                        nc.tensor.matmul(
                            psO[:, lo:TB],
                            v_s[:, j * HA : (j + 1) * HA],
                            pj[:, lo:TB],
                            start=(j == 0), stop=(j == nsc_i - 1),
                        )
                        if i == NBLK - 1 and j >= nsc_i - 4:
                            # final block: drain each psO stripe as its last
                            # PV lands so the out DMA overlaps remaining work
                            p = j - (nsc_i - 4)
                            sl = slice(p * 128, (p + 1) * 128)
                            nc.vector.tensor_copy(out_sb_last[:, sl], psO[:, sl])
                            oeng = nc.gpsimd if (F_OSPL and p % 2 == 1) else nc.sync
                            oeng.dma_start(
                                outT_d[:, q0 + p * 128 : q0 + (p + 1) * 128],
                                out_sb_last[:, sl],
                            )
                if i < NBLK - 1:
                    out_sb = out_pool.tile([HA, TB], F32)
                    if F_ENG:
                        nc.vector.tensor_copy(out_sb[:], psO[:])
                    else:
                        nc.scalar.copy(out_sb[:], psO[:])
                    nc.gpsimd.dma_start(outT_d[:, q0 : q0 + TB], out_sb[:])

    nc.compile()
    return nc


def _get_nc():
    if "nc" not in _compiled:
        _compiled["nc"] = build_nc()
    return _compiled["nc"]


def make_in_maps(x, Wk, Wq, Wv):
    x = np.asarray(x, dtype=np.float32)
    Wk = np.asarray(Wk, dtype=np.float32)
    Wq = np.asarray(Wq, dtype=np.float32)
    Wv = np.asarray(Wv, dtype=np.float32)
    # raw Wq (no 1/sqrt(C) here — that scale rides the exp's affine pre-scale)
    wqk = np.concatenate([Wq, Wk], axis=1).astype(np.float16)  # [C, 128]
    if F_WDIET:
        wvd = Wv.astype(np.float16)  # [C, 64]; duplicated on-chip
    else:
        wvd = np.concatenate([Wv, Wv], axis=1).astype(np.float16)  # [C, 128]
    triw = 129 if (F_WDIET and not F_MASK) else 385
    tri = np.zeros((128, triw), dtype=np.float16)
    tri[:, 0:128] = np.triu(np.ones((128, 128), dtype=np.float16))
    tri[:, 128] = 1.0
    if triw > 129:
        tri[:, 129:257] = np.triu(np.ones((128, 128), dtype=np.float16), k=1)
        tri[:, 257:385] = (-30000.0 * np.eye(128)).astype(np.float16)
    in_maps = []
    for b in range(B):
        in_maps.append(
            {
                "xT": np.ascontiguousarray(x[b].T.astype(np.float16)),
                "wqk": wqk,
                "wv": wvd,
                "tri": tri,
            }
        )
    return in_maps


def postprocess(results):
    outs = []
    for b in range(B):
        outT = results[b]["outT"]  # [65, T]
        out = (outT[:H] / outT[H : H + 1]).T  # [T, H]
        outs.append(out)
    return np.stack(outs).astype(np.float32)


def run(x, Wk, Wq, Wv, trace=False, **kw):
    nc = _get_nc()
    in_maps = make_in_maps(x, Wk, Wq, Wv)
    res = run_bass_kernel_spmd(
        nc, in_maps, core_ids=list(range(B)), trace=trace, **kw
    )
    return postprocess(res.results), res


def kernel(x, Wk, Wq, Wv):
    out, _ = run(x, Wk, Wq, Wv, trace=False)
    return out
